# revision 12
# baseline (speedup 1.0000x reference)
"""Causal self-attention (B=4, S=2048, D=1024, single head, fp32) on 8 trn2
NeuronCores.

Sharding: core 2*b + c handles batch b with the parity-c half of the keys
(global key rows 2*i + c), over ALL queries — a flash-attention split over
the key dimension. Each core returns unnormalized softmax numerators
o = sum_k exp(s~ - m~) v plus per-row stats (m = raw-score row max,
l = sum exp); the host combines the two key-halves exactly.

SPMD trick: one program serves both parities. The host pair-swaps the rows
of x for odd cores (rows [1,0,3,2,...]), so each core's keys sit at even
row positions and the on-chip stride-2 access pattern is parity-free. The
causal boundary masks (which depend on the parity) ship as a small
per-core input; the host pair-swaps the outputs of odd cores back.

Schedule: x^T is built in two halves. The lo half uses PE transposes (the
PE is otherwise idle at kernel start and the plain row DMA is fast); the
hi half uses a 32x32-block-permuting DMA plus DVE stream transposes, and
overlaps with the lo-half projections. Projections and attention run in
float32r (full PE rate at N=512, ~16x more accurate than bf16); attn @ v
runs in bf16. The attention loop is software-pipelined: query block j's
scores are issued before block j-1's attn@v so the softmax latency hides
under PE work.
"""
import math
import numpy as np

import concourse.bacc as bacc
import concourse.mybir as mybir
from concourse import tile
from concourse.masks import make_identity
from concourse.bass_utils import run_bass_kernel_spmd

B, S, D = 4, 2048, 1024
P = 128
DT = D // P          # 8 d-tiles (contraction)
ET = D // P          # 8 e-tiles (output feature)
ST = S // P          # 16 s-tiles (sequence)
HKT = ST // 2        # 8 compacted key tiles per core
NQB = S // P         # 16 query blocks
INV_SQRT_D = 1.0 / math.sqrt(D)
NEG = -1e30

F32 = mybir.dt.float32
F32R = mybir.dt.float32r
BF16 = mybir.dt.bfloat16

_CACHED_NC = None


def _ceil_div(a, b):
    return (a + b - 1) // b


def build_nc():
    nc = bacc.Bacc("TRN2", target_bir_lowering=False)
    x_p = nc.declare_dram_parameter("x", [S, D], F32, isOutput=False)
    wq_p = nc.declare_dram_parameter("wq", [D, D], F32, isOutput=False)
    wk_p = nc.declare_dram_parameter("wk", [D, D], F32, isOutput=False)
    wv_p = nc.declare_dram_parameter("wv", [D, D], F32, isOutput=False)
    mask_p = nc.declare_dram_parameter("mask", [P, 2, P], F32, isOutput=False)
    o_p = nc.declare_dram_parameter("o", [S, D], F32, isOutput=True)
    m_p = nc.declare_dram_parameter("m", [S, 1], F32, isOutput=True)
    l_p = nc.declare_dram_parameter("l", [S, 1], F32, isOutput=True)

    with tile.TileContext(nc) as tc:
        with (
            tc.tile_pool(name="qT_pool", bufs=1) as qT_pool,
            tc.tile_pool(name="kT_pool", bufs=1) as kT_pool,
            tc.tile_pool(name="v_pool", bufs=1) as v_pool,
            tc.tile_pool(name="const_pool", bufs=1) as const_pool,
        ):
            qT = qT_pool.tile([P, ET, S], F32R)        # [e_p, et, s_q] 64KB/p
            kT = kT_pool.tile([P, ET, HKT * P], F32R)  # [e_p, et, s_k] 32KB/p
            vv = v_pool.tile([P, HKT, D], BF16)        # [s_k_p, st, e] 16KB/p
            ident_f32 = const_pool.tile([P, P], F32)
            ident_bf = const_pool.tile([P, P], BF16)
            mask_sb = const_pool.tile([P, 2, P], F32)
            make_identity(nc, ident_f32[:])
            make_identity(nc, ident_bf[:])
            nc.sync.dma_start(out=mask_sb[:], in_=mask_p[:])

            # ================= Phase A: x^T + projections =================
            # x^T is split into lo (s-tiles 0..7) and hi (8..15) tiles so
            # lo-half projections can start while the hi half streams in.
            with (
                tc.tile_pool(name="xT_pool", bufs=1) as xT_pool,
                tc.tile_pool(name="stage_pool", bufs=2) as stage_pool,
                tc.tile_pool(name="psA_all", bufs=1, space="PSUM") as psAll,
            ):
                HS = ST // 2 * P                       # 1024 cols per half
                xTh = [xT_pool.tile([P, DT, HS], F32R, name=f"xT{h}")
                       for h in range(2)]
                psb = [psAll.tile([P, 512], F32, tag=f"b{i}", name=f"psb{i}")
                       for i in range(8)]

                # --- A1-lo: PE transposes (fast DMA, fills idle PE) ---
                for st in range(ST // 2):
                    x_f32 = stage_pool.tile([P, D], F32, tag="xs",
                                            name=f"xlo{st}", bufs=2)
                    nc.sync.dma_start(
                        out=x_f32[:], in_=x_p[st * P:(st + 1) * P, :])
                    for dt in range(DT):
                        ps = psb[(st * DT + dt) % 8]
                        nc.tensor.transpose(
                            ps[:, :P], x_f32[:, dt * P:(dt + 1) * P],
                            ident_f32[:])
                        nc.vector.tensor_copy(
                            xTh[0][:, dt, st * P:(st + 1) * P], ps[:, :P])

                def a1hi(st):
                    # block-permute DMA + DVE stream transpose, no PE, no PSUM
                    y_t = stage_pool.tile([P, DT, P], F32, tag="ys",
                                          name=f"ys{st}", bufs=2)
                    z_t = stage_pool.tile([P, DT, P], F32, tag="zs",
                                          name=f"zs{st}", bufs=1)
                    x_r = x_p[(ST // 2 + st) * P:(ST // 2 + st + 1) * P, :]\
                        .rearrange("(b w) (dt a u) -> a w dt b u",
                                   b=4, w=32, dt=DT, a=4, u=32)
                    for a in range(4):
                        nc.sync.dma_start(
                            out=y_t[32 * a:32 * (a + 1), :, :].rearrange(
                                "w dt (b u) -> w dt b u", b=4),
                            in_=x_r[a])
                    for dt in range(DT):
                        nc.vector.transpose(z_t[:, dt, :], y_t[:, dt, :])
                    # rounding fp32 -> fp32r (required by the PE verifier)
                    nc.vector.tensor_copy(
                        xTh[1][:, :, st * P:(st + 1) * P], z_t[:])

                # per-half stride-2 views (this core's keys at even positions)
                xk = [xTh[h].rearrange("p d (s two) -> p d two s", two=2)
                      for h in range(2)]

                def a2(h, et):
                    ps = psb[et % 2]
                    for dh in range(2):
                        wk_f = stage_pool.tile([P, 4, P], F32, tag="wf",
                                               name=f"wkf{h}_{et}_{dh}")
                        wk_r = stage_pool.tile([P, 4, P], F32R, tag="wr",
                                               name=f"wkr{h}_{et}_{dh}")
                        nc.sync.dma_start(
                            out=wk_f[:],
                            in_=wk_p[dh * 512:(dh + 1) * 512,
                                     et * P:(et + 1) * P].rearrange(
                                "(dt p) e -> p dt e", p=P))
                        nc.vector.tensor_copy(wk_r[:], wk_f[:])
                        for d4 in range(4):
                            d = dh * 4 + d4
                            nc.tensor.matmul(
                                ps[:], wk_r[:, d4, :], xk[h][:, d, 0, :],
                                start=(d == 0), stop=(d == DT - 1))
                    nc.vector.tensor_copy(
                        kT[:, et, h * 512:(h + 1) * 512], ps[:])

                def a4(h, et):
                    pss = [psb[2 + ch * 2 + (et % 2)] for ch in range(2)]
                    for dh in range(2):
                        wq_f = stage_pool.tile([P, 4, P], F32, tag="wf",
                                               name=f"wqf{h}_{et}_{dh}")
                        wq_r = stage_pool.tile([P, 4, P], F32R, tag="wr",
                                               name=f"wqr{h}_{et}_{dh}")
                        nc.sync.dma_start(
                            out=wq_f[:],
                            in_=wq_p[dh * 512:(dh + 1) * 512,
                                     et * P:(et + 1) * P].rearrange(
                                "(dt p) e -> p dt e", p=P))
                        nc.vector.tensor_copy(wq_r[:], wq_f[:])
                        for d4 in range(4):
                            d = dh * 4 + d4
                            for ch in range(2):
                                nc.tensor.matmul(
                                    pss[ch][:],
                                    wq_r[:, d4, :],
                                    xTh[h][:, d, ch * 512:(ch + 1) * 512],
                                    start=(d == 0), stop=(d == DT - 1))
                    for ch in range(2):
                        nc.vector.tensor_copy(
                            qT[:, et, h * 1024 + ch * 512:
                               h * 1024 + (ch + 1) * 512], pss[ch][:])

                def a3(h, eb):
                    for d in range(DT):
                        wv_f = stage_pool.tile([P, 512], F32, tag="wf",
                                               name=f"wvf{h}_{eb}_{d}")
                        wv_r = stage_pool.tile([P, 512], F32R, tag="wr",
                                               name=f"wvr{h}_{eb}_{d}")
                        nc.sync.dma_start(
                            out=wv_f[:],
                            in_=wv_p[d * P:(d + 1) * P,
                                     eb * 512:(eb + 1) * 512])
                        nc.vector.tensor_copy(wv_r[:], wv_f[:])
                        for sq in range(4):
                            nc.tensor.matmul(
                                psb[4 * eb + sq][:],
                                xk[h][:, d, 0, sq * P:(sq + 1) * P],
                                wv_r[:],
                                start=(d == 0), stop=(d == DT - 1))
                    for sq in range(4):
                        nc.vector.tensor_copy(
                            vv[:, 4 * h + sq, eb * 512:(eb + 1) * 512],
                            psb[4 * eb + sq][:])

                # lo-half projections, with the hi-half x^T streaming in
                # during the PE-heavy, DVE-light A4 pass
                for et in range(ET):
                    a2(0, et)
                for et in range(ET):
                    a4(0, et)
                    a1hi(et)
                for eb in range(2):
                    a3(0, eb)
                for et in range(ET):
                    a2(1, et)
                for et in range(ET):
                    a4(1, et)
                for eb in range(2):
                    a3(1, eb)

            # ================= Phase B: causal attention =================
            # Software pipeline: scores/softmax for block j are issued before
            # attn@v for block j-1, so the softmax latency hides under PE work.
            with (
                tc.tile_pool(name="sc_pool", bufs=2) as sc_pool,
                tc.tile_pool(name="at_pool", bufs=2) as at_pool,
                tc.tile_pool(name="atT_pool", bufs=4) as atT_pool,
                tc.tile_pool(name="st_pool", bufs=4) as st_pool,
                tc.tile_pool(name="ob_pool", bufs=2) as ob_pool,
                tc.tile_pool(name="psS_pool", bufs=2, space="PSUM") as psS_pool,
                tc.tile_pool(name="psA_pool", bufs=2, space="PSUM") as psA_pool,
                tc.tile_pool(name="psO_pool", bufs=1, space="PSUM") as psO_pool,
            ):
                attn_tiles = {}

                def scores_softmax(j):
                    nkb = j // 2 + 1
                    ncols = nkb * P
                    nch = _ceil_div(ncols, 512)
                    scores = sc_pool.tile([P, HKT * P], F32, tag="scores",
                                          name=f"scores{j}")
                    attn = at_pool.tile([P, HKT * P], BF16, tag="attn",
                                        name=f"attn{j}")
                    for ch in range(nch):
                        ncc = min(512, ncols - ch * 512)
                        psS = psS_pool.tile([P, 512], F32, tag=f"psS{ch % 2}",
                                            name=f"psS{j}_{ch}")
                        for et in range(ET):
                            nc.tensor.matmul(
                                psS[:, :ncc],
                                qT[:, et, j * P:(j + 1) * P],
                                kT[:, et, ch * 512:ch * 512 + ncc],
                                start=(et == 0), stop=(et == ET - 1))
                        lo, hi = ch * 512, ch * 512 + ncc
                        if hi == ncols:
                            if ncc > P:
                                nc.vector.tensor_copy(scores[:, lo:hi - P],
                                                      psS[:, :ncc - P])
                            nc.vector.tensor_add(
                                scores[:, hi - P:hi],
                                psS[:, ncc - P:ncc],
                                mask_sb[:, j % 2, :])
                        else:
                            nc.vector.tensor_copy(scores[:, lo:hi],
                                                  psS[:, :ncc])

                    m_t = st_pool.tile([P, 1], F32, tag="m", name=f"m{j}")
                    neg_t = st_pool.tile([P, 1], F32, tag="neg", name=f"neg{j}")
                    l_t = st_pool.tile([P, 1], F32, tag="l", name=f"l{j}")
                    nc.vector.reduce_max(m_t[:], scores[:, :ncols],
                                         axis=mybir.AxisListType.X)
                    nc.vector.tensor_scalar_mul(neg_t[:], m_t[:], -INV_SQRT_D)
                    nc.scalar.activation(
                        attn[:, :ncols], scores[:, :ncols],
                        mybir.ActivationFunctionType.Exp,
                        bias=neg_t[:], scale=INV_SQRT_D, accum_out=l_t[:])
                    nc.sync.dma_start(out=m_p[j * P:(j + 1) * P, :], in_=m_t[:])
                    nc.sync.dma_start(out=l_p[j * P:(j + 1) * P, :], in_=l_t[:])
                    attn_tiles[j] = attn

                def att_v(j):
                    nkb = j // 2 + 1
                    attn = attn_tiles.pop(j)
                    atTs = []
                    for kb in range(nkb):
                        psA = psA_pool.tile([P, P], BF16, tag="psA",
                                            name=f"psA{j}_{kb}")
                        atT = atT_pool.tile([P, P], BF16, tag="atT",
                                            name=f"atT{j}_{kb}")
                        nc.tensor.transpose(
                            psA[:], attn[:, kb * P:(kb + 1) * P], ident_bf[:])
                        nc.vector.tensor_copy(atT[:], psA[:])
                        atTs.append(atT)
                    psO = [psO_pool.tile([P, 512], F32, tag=f"psO{eb}",
                                         name=f"psO{j}_{eb}")
                           for eb in range(2)]
                    for kb in range(nkb):
                        for eb in range(2):
                            nc.tensor.matmul(
                                psO[eb][:],
                                atTs[kb][:],
                                vv[:, kb, eb * 512:(eb + 1) * 512],
                                start=(kb == 0), stop=(kb == nkb - 1))
                    for eb in range(2):
                        o_sb = ob_pool.tile([P, 512], F32, tag="o",
                                            name=f"o{j}_{eb}")
                        nc.vector.tensor_copy(o_sb[:], psO[eb][:])
                        nc.sync.dma_start(
                            out=o_p[j * P:(j + 1) * P,
                                    eb * 512:(eb + 1) * 512],
                            in_=o_sb[:])

                for j in range(NQB):
                    scores_softmax(j)
                    if j > 0:
                        att_v(j - 1)
                att_v(NQB - 1)
    nc.finalize()
    return nc


def _boundary_masks(c):
    """mask[row, par, i]: 0 if compacted key i is causally valid for local
    query row `row` of an even (par=0) / odd (par=1) query block, else -1e30.

    For parity-1 cores, x rows arrive pair-swapped, so the query at local
    position `row` is global row 128*j + r_local with
    r_local = row+1 (even row) / row-1 (odd row). Key i is global row
    256*(j//2) + 2*i + c. Valid iff 2*i + c <= par*128 + r_local.
    """
    mask = np.full((P, 2, P), NEG, dtype=np.float32)
    for row in range(P):
        r_local = row if c == 0 else (row + 1 if row % 2 == 0 else row - 1)
        for par in range(2):
            lim = (par * P + r_local - c) // 2
            if lim >= 0:
                mask[row, par, :min(lim + 1, P)] = 0.0
    return mask


_PAIRSWAP = np.arange(S).reshape(-1, 2)[:, ::-1].reshape(-1)


def _make_in_maps(x, Wq, Wk, Wv):
    x = np.asarray(x, dtype=np.float32)
    Wq = np.ascontiguousarray(np.asarray(Wq, dtype=np.float32))
    Wk = np.ascontiguousarray(np.asarray(Wk, dtype=np.float32))
    Wv = np.ascontiguousarray(np.asarray(Wv, dtype=np.float32))
    masks = [_boundary_masks(0), _boundary_masks(1)]
    in_maps = []
    for core in range(8):
        b, c = core // 2, core % 2
        xb = x[b] if c == 0 else x[b][_PAIRSWAP]
        in_maps.append({
            "x": np.ascontiguousarray(xb),
            "wq": Wq, "wk": Wk, "wv": Wv,
            "mask": masks[c],
        })
    return in_maps


def _combine(res):
    out = np.empty((B, S, D), dtype=np.float32)
    for b in range(B):
        r0, r1 = res.results[2 * b], res.results[2 * b + 1]
        o0, m0, l0 = r0["o"], r0["m"], r0["l"]
        # parity-1 core computed on pair-swapped query rows; swap back
        o1 = r1["o"][_PAIRSWAP]
        m1 = r1["m"][_PAIRSWAP]
        l1 = r1["l"][_PAIRSWAP]
        ms0 = m0.astype(np.float64) * INV_SQRT_D
        ms1 = m1.astype(np.float64) * INV_SQRT_D
        mm = np.maximum(ms0, ms1)
        w0 = np.exp(ms0 - mm)
        w1 = np.exp(ms1 - mm)
        num = w0 * o0.astype(np.float64) + w1 * o1.astype(np.float64)
        den = w0 * l0.astype(np.float64) + w1 * l1.astype(np.float64)
        out[b] = (num / den).astype(np.float32)
    return out


def kernel(x, Wq, Wk, Wv):
    global _CACHED_NC
    if _CACHED_NC is None:
        _CACHED_NC = build_nc()
    in_maps = _make_in_maps(x, Wq, Wk, Wv)
    res = run_bass_kernel_spmd(_CACHED_NC, in_maps, list(range(8)))
    return _combine(res)


# revision 13
# speedup vs baseline: 1.3978x; 1.3978x over previous
"""Causal self-attention (B=4, S=2048, D=1024, single head, fp32) on 8 trn2
NeuronCores.

Sharding: core 2*b + c handles batch b with the parity-c half of the keys
(global key rows 2*i + c), over ALL queries — a flash-attention split over
the key dimension. Each core returns unnormalized softmax numerators
o = sum_k exp(s~ - m~) v plus per-row stats (m = raw-score row max,
l = sum exp); the host combines the two key-halves exactly.

SPMD trick: one program serves both parities. The host pair-swaps the rows
of x for odd cores (rows [1,0,3,2,...]), so each core's keys sit at even
row positions and the on-chip stride-2 access pattern is parity-free. The
causal boundary masks (which depend on the parity) ship as a small
per-core input; the host pair-swaps the outputs of odd cores back.

Schedule: x^T is built in two halves. The lo half uses PE transposes (the
PE is otherwise idle at kernel start and the plain row DMA is fast); the
hi half uses a 32x32-block-permuting DMA plus DVE stream transposes, and
overlaps with the lo-half projections. Projections and attention run in
float32r (full PE rate at N=512, ~16x more accurate than bf16); attn @ v
runs in bf16. The attention loop is software-pipelined: query block j's
scores are issued before block j-1's attn@v so the softmax latency hides
under PE work.
"""
import math
import numpy as np

import concourse.bacc as bacc
import concourse.mybir as mybir
from concourse import tile
from concourse.masks import make_identity
from concourse.bass_utils import run_bass_kernel_spmd

B, S, D = 4, 2048, 1024
P = 128
DT = D // P          # 8 d-tiles (contraction)
ET = D // P          # 8 e-tiles (output feature)
ST = S // P          # 16 s-tiles (sequence)
HKT = ST // 2        # 8 compacted key tiles per core
NQB = S // P         # 16 query blocks
INV_SQRT_D = 1.0 / math.sqrt(D)
NEG = -1e30

F32 = mybir.dt.float32
F32R = mybir.dt.float32r
BF16 = mybir.dt.bfloat16

_CACHED_NC = None


def _ceil_div(a, b):
    return (a + b - 1) // b


def build_nc():
    nc = bacc.Bacc("TRN2", target_bir_lowering=False)
    x_p = nc.declare_dram_parameter("x", [S, D], F32, isOutput=False)
    wq_p = nc.declare_dram_parameter("wq", [D, D], F32, isOutput=False)
    wk_p = nc.declare_dram_parameter("wk", [D, D], F32, isOutput=False)
    wv_p = nc.declare_dram_parameter("wv", [D, D], F32, isOutput=False)
    mask_p = nc.declare_dram_parameter("mask", [P, 2, P], F32, isOutput=False)
    o_p = nc.declare_dram_parameter("o", [S, D], F32, isOutput=True)
    m_p = nc.declare_dram_parameter("m", [P, NQB], F32, isOutput=True)
    l_p = nc.declare_dram_parameter("l", [P, NQB], F32, isOutput=True)

    with tile.TileContext(nc) as tc:
        with (
            tc.tile_pool(name="qT_pool", bufs=1) as qT_pool,
            tc.tile_pool(name="kT_pool", bufs=1) as kT_pool,
            tc.tile_pool(name="v_pool", bufs=1) as v_pool,
            tc.tile_pool(name="const_pool", bufs=1) as const_pool,
        ):
            qT = qT_pool.tile([P, ET, S], F32R)        # [e_p, et, s_q] 64KB/p
            kT = kT_pool.tile([P, ET, HKT * P], F32R)  # [e_p, et, s_k] 32KB/p
            vv = v_pool.tile([P, HKT, D], BF16)        # [s_k_p, st, e] 16KB/p
            ident_f32 = const_pool.tile([P, P], F32)
            ident_bf = const_pool.tile([P, P], BF16)
            mask_sb = const_pool.tile([P, 2, P], F32)
            m_all = const_pool.tile([P, NQB], F32)
            l_all = const_pool.tile([P, NQB], F32)
            make_identity(nc, ident_f32[:])
            make_identity(nc, ident_bf[:])
            nc.sync.dma_start(out=mask_sb[:], in_=mask_p[:])

            # ================= Phase A: x^T + projections =================
            # x^T is split into lo (s-tiles 0..7) and hi (8..15) tiles so
            # lo-half projections can start while the hi half streams in.
            with (
                tc.tile_pool(name="xT_pool", bufs=1) as xT_pool,
                tc.tile_pool(name="stage_pool", bufs=2) as stage_pool,
                tc.tile_pool(name="psA_all", bufs=1, space="PSUM") as psAll,
            ):
                HS = ST // 2 * P                       # 1024 cols per half
                xTh = [xT_pool.tile([P, DT, HS], F32R, name=f"xT{h}")
                       for h in range(2)]
                psb = [psAll.tile([P, 512], F32, tag=f"b{i}", name=f"psb{i}")
                       for i in range(8)]

                # --- A1-lo: PE transposes (fast DMA, fills idle PE) ---
                for st in range(ST // 2):
                    x_f32 = stage_pool.tile([P, D], F32, tag="xs",
                                            name=f"xlo{st}", bufs=2)
                    nc.sync.dma_start(
                        out=x_f32[:], in_=x_p[st * P:(st + 1) * P, :])
                    for dt in range(DT):
                        ps = psb[(st * DT + dt) % 8]
                        nc.tensor.transpose(
                            ps[:, :P], x_f32[:, dt * P:(dt + 1) * P],
                            ident_f32[:])
                        nc.vector.tensor_copy(
                            xTh[0][:, dt, st * P:(st + 1) * P], ps[:, :P])

                def a1hi(st):
                    # block-permute DMA + DVE stream transpose, no PE, no PSUM
                    y_t = stage_pool.tile([P, DT, P], F32, tag="ys",
                                          name=f"ys{st}", bufs=2)
                    z_t = stage_pool.tile([P, DT, P], F32, tag="zs",
                                          name=f"zs{st}", bufs=1)
                    x_r = x_p[(ST // 2 + st) * P:(ST // 2 + st + 1) * P, :]\
                        .rearrange("(b w) (dt a u) -> a w dt b u",
                                   b=4, w=32, dt=DT, a=4, u=32)
                    for a in range(4):
                        nc.sync.dma_start(
                            out=y_t[32 * a:32 * (a + 1), :, :].rearrange(
                                "w dt (b u) -> w dt b u", b=4),
                            in_=x_r[a])
                    for dt in range(DT):
                        nc.vector.transpose(z_t[:, dt, :], y_t[:, dt, :])
                    # rounding fp32 -> fp32r (required by the PE verifier)
                    nc.gpsimd.tensor_copy(
                        xTh[1][:, :, st * P:(st + 1) * P], z_t[:])

                # per-half stride-2 views (this core's keys at even positions)
                xk = [xTh[h].rearrange("p d (s two) -> p d two s", two=2)
                      for h in range(2)]

                def a2(h, et):
                    ps = psb[et % 2]
                    for dh in range(2):
                        wk_f = stage_pool.tile([P, 4, P], F32, tag="wf",
                                               name=f"wkf{h}_{et}_{dh}")
                        wk_r = stage_pool.tile([P, 4, P], F32R, tag="wr",
                                               name=f"wkr{h}_{et}_{dh}")
                        nc.sync.dma_start(
                            out=wk_f[:],
                            in_=wk_p[dh * 512:(dh + 1) * 512,
                                     et * P:(et + 1) * P].rearrange(
                                "(dt p) e -> p dt e", p=P))
                        nc.gpsimd.tensor_copy(wk_r[:], wk_f[:])
                        for d4 in range(4):
                            d = dh * 4 + d4
                            nc.tensor.matmul(
                                ps[:], wk_r[:, d4, :], xk[h][:, d, 0, :],
                                start=(d == 0), stop=(d == DT - 1))
                    nc.vector.tensor_copy(
                        kT[:, et, h * 512:(h + 1) * 512], ps[:])

                def a4(h, et):
                    pss = [psb[2 + ch * 2 + (et % 2)] for ch in range(2)]
                    for dh in range(2):
                        wq_f = stage_pool.tile([P, 4, P], F32, tag="wf",
                                               name=f"wqf{h}_{et}_{dh}")
                        wq_r = stage_pool.tile([P, 4, P], F32R, tag="wr",
                                               name=f"wqr{h}_{et}_{dh}")
                        nc.sync.dma_start(
                            out=wq_f[:],
                            in_=wq_p[dh * 512:(dh + 1) * 512,
                                     et * P:(et + 1) * P].rearrange(
                                "(dt p) e -> p dt e", p=P))
                        nc.gpsimd.tensor_copy(wq_r[:], wq_f[:])
                        for d4 in range(4):
                            d = dh * 4 + d4
                            for ch in range(2):
                                nc.tensor.matmul(
                                    pss[ch][:],
                                    wq_r[:, d4, :],
                                    xTh[h][:, d, ch * 512:(ch + 1) * 512],
                                    start=(d == 0), stop=(d == DT - 1))
                    for ch in range(2):
                        nc.vector.tensor_copy(
                            qT[:, et, h * 1024 + ch * 512:
                               h * 1024 + (ch + 1) * 512], pss[ch][:])

                def a3(h, eb):
                    for d in range(DT):
                        wv_f = stage_pool.tile([P, 512], F32, tag="wf",
                                               name=f"wvf{h}_{eb}_{d}")
                        wv_r = stage_pool.tile([P, 512], F32R, tag="wr",
                                               name=f"wvr{h}_{eb}_{d}")
                        nc.sync.dma_start(
                            out=wv_f[:],
                            in_=wv_p[d * P:(d + 1) * P,
                                     eb * 512:(eb + 1) * 512])
                        nc.gpsimd.tensor_copy(wv_r[:], wv_f[:])
                        for sq in range(4):
                            nc.tensor.matmul(
                                psb[4 * eb + sq][:],
                                xk[h][:, d, 0, sq * P:(sq + 1) * P],
                                wv_r[:],
                                start=(d == 0), stop=(d == DT - 1))
                    for sq in range(4):
                        nc.vector.tensor_copy(
                            vv[:, 4 * h + sq, eb * 512:(eb + 1) * 512],
                            psb[4 * eb + sq][:])

                # lo-half projections, with the hi-half x^T streaming in
                # during the PE-heavy, DVE-light A4 pass
                for et in range(ET):
                    a2(0, et)
                for et in range(ET):
                    a4(0, et)
                    a1hi(et)
                for eb in range(2):
                    a3(0, eb)
                for et in range(ET):
                    a2(1, et)
                for et in range(ET):
                    a4(1, et)
                for eb in range(2):
                    a3(1, eb)

            # ================= Phase B: causal attention =================
            # Software pipeline: scores/softmax for block j are issued before
            # attn@v for block j-1, so the softmax latency hides under PE work.
            with (
                tc.tile_pool(name="sc_pool", bufs=2) as sc_pool,
                tc.tile_pool(name="at_pool", bufs=2) as at_pool,
                tc.tile_pool(name="atT_pool", bufs=4) as atT_pool,
                tc.tile_pool(name="st_pool", bufs=4) as st_pool,
                tc.tile_pool(name="ob_pool", bufs=3) as ob_pool,
                tc.tile_pool(name="psS_pool", bufs=2, space="PSUM") as psS_pool,
                tc.tile_pool(name="psA_pool", bufs=2, space="PSUM") as psA_pool,
                tc.tile_pool(name="psO_pool", bufs=1, space="PSUM") as psO_pool,
            ):
                attn_tiles = {}

                def scores_softmax(j):
                    nkb = j // 2 + 1
                    ncols = nkb * P
                    nch = _ceil_div(ncols, 512)
                    scores = sc_pool.tile([P, HKT * P], F32, tag="scores",
                                          name=f"scores{j}")
                    attn = at_pool.tile([P, HKT * P], BF16, tag="attn",
                                        name=f"attn{j}")
                    for ch in range(nch):
                        ncc = min(512, ncols - ch * 512)
                        psS = psS_pool.tile([P, 512], F32, tag=f"psS{ch % 2}",
                                            name=f"psS{j}_{ch}")
                        for et in range(ET):
                            nc.tensor.matmul(
                                psS[:, :ncc],
                                qT[:, et, j * P:(j + 1) * P],
                                kT[:, et, ch * 512:ch * 512 + ncc],
                                start=(et == 0), stop=(et == ET - 1))
                        lo, hi = ch * 512, ch * 512 + ncc
                        if hi == ncols:
                            if ncc > P:
                                nc.vector.tensor_copy(scores[:, lo:hi - P],
                                                      psS[:, :ncc - P])
                            nc.vector.tensor_add(
                                scores[:, hi - P:hi],
                                psS[:, ncc - P:ncc],
                                mask_sb[:, j % 2, :])
                        else:
                            nc.vector.tensor_copy(scores[:, lo:hi],
                                                  psS[:, :ncc])

                    neg_t = st_pool.tile([P, 1], F32, tag="neg", name=f"neg{j}")
                    nc.vector.reduce_max(m_all[:, j:j + 1], scores[:, :ncols],
                                         axis=mybir.AxisListType.X)
                    nc.vector.tensor_scalar_mul(neg_t[:], m_all[:, j:j + 1],
                                                -INV_SQRT_D)
                    nc.scalar.activation(
                        attn[:, :ncols], scores[:, :ncols],
                        mybir.ActivationFunctionType.Exp,
                        bias=neg_t[:], scale=INV_SQRT_D,
                        accum_out=l_all[:, j:j + 1])
                    attn_tiles[j] = attn

                def att_v(j):
                    nkb = j // 2 + 1
                    attn = attn_tiles.pop(j)
                    atTs = []
                    for kb in range(nkb):
                        psA = psA_pool.tile([P, P], BF16, tag="psA",
                                            name=f"psA{j}_{kb}")
                        atT = atT_pool.tile([P, P], BF16, tag="atT",
                                            name=f"atT{j}_{kb}")
                        nc.tensor.transpose(
                            psA[:], attn[:, kb * P:(kb + 1) * P], ident_bf[:])
                        nc.vector.tensor_copy(atT[:], psA[:])
                        atTs.append(atT)
                    psO = [psO_pool.tile([P, 512], F32, tag=f"psO{eb}",
                                         name=f"psO{j}_{eb}")
                           for eb in range(2)]
                    for kb in range(nkb):
                        for eb in range(2):
                            nc.tensor.matmul(
                                psO[eb][:],
                                atTs[kb][:],
                                vv[:, kb, eb * 512:(eb + 1) * 512],
                                start=(kb == 0), stop=(kb == nkb - 1))
                    for eb in range(2):
                        o_sb = ob_pool.tile([P, 512], F32, tag="o",
                                            name=f"o{j}_{eb}")
                        nc.vector.tensor_copy(o_sb[:], psO[eb][:])
                        nc.sync.dma_start(
                            out=o_p[j * P:(j + 1) * P,
                                    eb * 512:(eb + 1) * 512],
                            in_=o_sb[:])

                for j in range(NQB):
                    scores_softmax(j)
                    if j > 0:
                        att_v(j - 1)
                att_v(NQB - 1)
                nc.sync.dma_start(out=m_p[:], in_=m_all[:])
                nc.sync.dma_start(out=l_p[:], in_=l_all[:])
    nc.finalize()
    return nc


def _boundary_masks(c):
    """mask[row, par, i]: 0 if compacted key i is causally valid for local
    query row `row` of an even (par=0) / odd (par=1) query block, else -1e30.

    For parity-1 cores, x rows arrive pair-swapped, so the query at local
    position `row` is global row 128*j + r_local with
    r_local = row+1 (even row) / row-1 (odd row). Key i is global row
    256*(j//2) + 2*i + c. Valid iff 2*i + c <= par*128 + r_local.
    """
    mask = np.full((P, 2, P), NEG, dtype=np.float32)
    for row in range(P):
        r_local = row if c == 0 else (row + 1 if row % 2 == 0 else row - 1)
        for par in range(2):
            lim = (par * P + r_local - c) // 2
            if lim >= 0:
                mask[row, par, :min(lim + 1, P)] = 0.0
    return mask


_PAIRSWAP = np.arange(S).reshape(-1, 2)[:, ::-1].reshape(-1)


def _make_in_maps(x, Wq, Wk, Wv):
    x = np.asarray(x, dtype=np.float32)
    Wq = np.ascontiguousarray(np.asarray(Wq, dtype=np.float32))
    Wk = np.ascontiguousarray(np.asarray(Wk, dtype=np.float32))
    Wv = np.ascontiguousarray(np.asarray(Wv, dtype=np.float32))
    masks = [_boundary_masks(0), _boundary_masks(1)]
    in_maps = []
    for core in range(8):
        b, c = core // 2, core % 2
        xb = x[b] if c == 0 else x[b][_PAIRSWAP]
        in_maps.append({
            "x": np.ascontiguousarray(xb),
            "wq": Wq, "wk": Wk, "wv": Wv,
            "mask": masks[c],
        })
    return in_maps


def _combine(res):
    out = np.empty((B, S, D), dtype=np.float32)
    for b in range(B):
        r0, r1 = res.results[2 * b], res.results[2 * b + 1]
        o0 = r0["o"]
        # parity-1 core computed on pair-swapped query rows; swap back
        def stat(r, key):
            return np.ascontiguousarray(r[key].T).reshape(S, 1)
        m0, l0 = stat(r0, "m"), stat(r0, "l")
        o1 = r1["o"][_PAIRSWAP]
        m1 = stat(r1, "m")[_PAIRSWAP]
        l1 = stat(r1, "l")[_PAIRSWAP]
        ms0 = m0.astype(np.float64) * INV_SQRT_D
        ms1 = m1.astype(np.float64) * INV_SQRT_D
        mm = np.maximum(ms0, ms1)
        w0 = np.exp(ms0 - mm)
        w1 = np.exp(ms1 - mm)
        num = w0 * o0.astype(np.float64) + w1 * o1.astype(np.float64)
        den = w0 * l0.astype(np.float64) + w1 * l1.astype(np.float64)
        out[b] = (num / den).astype(np.float32)
    return out


def kernel(x, Wq, Wk, Wv):
    global _CACHED_NC
    if _CACHED_NC is None:
        _CACHED_NC = build_nc()
    in_maps = _make_in_maps(x, Wq, Wk, Wv)
    res = run_bass_kernel_spmd(_CACHED_NC, in_maps, list(range(8)))
    return _combine(res)
